# revision 17
# baseline (speedup 1.0000x reference)
"""t-SNE style probability encoder on 8 trn2 cores — v4.

Baseline phase-1 (own-block fp32 MLP + AllGather of bf16 hi/lo z
splits) combined with a rebuilt phase 2:

  R_k tiles [52, 1024] (one per 1024 gathered cols):
    rows [0:16]=zhi, [16:32]=zhi(dup), [32:48]=zlo, [48]=sqh,
    [49]=sql, [50:52]=1   (scattered from the AllGather output)
  L [52, 1024] (own points): rows [0:16]=-2zhi, [16:32]=-2zlo,
    [32:48]=-2zhi, [48:50]=1, [50]=sqp1h, [51]=sqp1l
  => matmul(L.T, R) = 1 + sq_i + sq_j - 2 z_i.z_j = 1 + dist

Phase 2 per 128-row m-block splits columns between two engine paths:
  ACT path (cols [0:CA)):  Ln (PSUM->SBUF fp32) then Exp(scale=-1)
        -> f16 with fused rowsum accum:  exp(-ln(1+d)) = 1/(1+d).
  DVE path (cols [CA:N)):  custom 7-stage DVE op RECIP_CUBIC_ACC_ANT:
        y = ~x * (C0*u^2 + C1*u + C2), u = x*~x  (bitwise-NOT
        exponent-flip seed + minimax quadratic, ~5e-5 rel err),
        f16 out with fused rowsum accum.
The normalize (tensor_scalar f16 4x) + output DMA for block m-1 are
issued while block m computes (software pipelining), so the DVE queue
never stalls behind a rowsum that isn't ready. gpsimd does no
elementwise work (the baseline's normalize_recip was its bottleneck).
"""

import sys

import numpy as np

sys.path.insert(0, "/opt/trn_rl_repo")

N = 8192
DIM = 128
EMB = 16
NCORES = 8
ROWS = N // NCORES  # 1024
KAUG = 52
GROWS = 34  # gathered rows per core: zhi(16) zlo(16) sqh sql
NK = N // 1024

CA = 3072  # ACT-path columns per m-block; DVE path gets N - CA

RECIP_C0 = -0.013060557583875289
RECIP_C1 = -0.16652211161246575
RECIP_C2 = -0.7071067706768506

_CACHE = {}


def _ensure_custom_op():
    """Register RECIP_CUBIC_ACC_ANT (7-stage approx recip + rowsum accum)."""
    from operator import add

    import concourse.dve_ops as dve_ops
    from concourse.dve_spec import AluOp, Bin, C0, C1, C2, Spec, Src0, Zero

    if "RECIP_CUBIC_ACC_ANT" in dve_ops._SUB_OPCODE_FOR_NAME:
        return next(o for o in dve_ops.OPS if o.name == "RECIP_CUBIC_ACC_ANT")

    _nx = Bin(AluOp.BITWISE_NOT, Src0, Src0)
    _u = Src0 * _nx
    _body = _nx * ((_u * C0 + C1) * _u + C2)

    def _ref(in0, in1, c0, c1, c2):
        f32 = np.float32
        x = np.asarray(in0, f32)
        nx = (~x.view(np.int32)).view(f32)
        u = f32(x * nx)
        p = (u * f32(c0) + f32(c1)) * u + f32(c2)
        y = (nx * p).astype(f32)
        return y, np.sum(y, axis=-1, keepdims=True, dtype=f32)

    op = dve_ops.DveOp(
        "RECIP_CUBIC_ACC_ANT",
        Spec(body=_body, accum=add, accum_init=Zero, reference=_ref),
        subdim=False,
        uops_sha={"v3": "055837844467d4fd", "v4": "427f29ef65565b5a"},
    )
    row = max(dve_ops._SUB_OPCODE_FOR_NAME.values()) + 1
    assert row < 0x20, "no free custom-DVE opcode row"
    dve_ops.OPS.append(op)
    dve_ops.CUSTOM_DVE_SPECS[op.name] = op.spec
    dve_ops._SUB_OPCODE_FOR_NAME[op.name] = row
    return op


def _build_program():
    from contextlib import ExitStack

    import concourse.bacc as bacc
    import concourse.tile as tile
    from concourse import mybir

    recip_op = _ensure_custom_op()

    f32 = mybir.dt.float32
    f16 = mybir.dt.float16
    bf16 = mybir.dt.bfloat16
    AF = mybir.ActivationFunctionType
    Alu = mybir.AluOpType

    nc = bacc.Bacc("TRN2", target_bir_lowering=False, debug=False, num_devices=NCORES)

    xT = nc.declare_dram_parameter("xT", [DIM, ROWS], f32, isOutput=False)
    W1 = nc.declare_dram_parameter("W1", [128, 64], f32, isOutput=False)
    W2 = nc.declare_dram_parameter("W2", [64, 32], f32, isOutput=False)
    W3 = nc.declare_dram_parameter("W3", [32, 16], f32, isOutput=False)
    W4 = nc.declare_dram_parameter("W4", [16, 16], f32, isOutput=False)
    b1 = nc.declare_dram_parameter("b1", [64, 1], f32, isOutput=False)
    b2 = nc.declare_dram_parameter("b2", [32, 1], f32, isOutput=False)
    b3 = nc.declare_dram_parameter("b3", [16, 1], f32, isOutput=False)
    b4 = nc.declare_dram_parameter("b4", [16, 1], f32, isOutput=False)
    out = nc.declare_dram_parameter("out", [ROWS, N], f16, isOutput=True)

    with tile.TileContext(nc) as tc, ExitStack() as ctx:
        consts = ctx.enter_context(tc.tile_pool(name="consts", bufs=1))
        persist = ctx.enter_context(tc.tile_pool(name="persist", bufs=1))
        dram = ctx.enter_context(tc.tile_pool(name="dram", bufs=1, space="DRAM"))

        xt_sb = consts.tile([DIM, ROWS], f32)
        w1_sb = consts.tile([128, 64], f32)
        w2_sb = consts.tile([64, 32], f32)
        w3_sb = consts.tile([32, 16], f32)
        w4_sb = consts.tile([16, 16], f32)
        b1_sb = consts.tile([64, 1], f32)
        b2_sb = consts.tile([32, 1], f32)
        b3_sb = consts.tile([16, 1], f32)
        b4_sb = consts.tile([16, 1], f32)
        ones_sq = consts.tile([16, 1], f32)
        for drm, sb in [
            (xT, xt_sb), (W1, w1_sb), (b1, b1_sb),
            (W2, w2_sb), (W3, w3_sb), (W4, w4_sb),
            (b2, b2_sb), (b3, b3_sb), (b4, b4_sb),
        ]:
            nc.sync.dma_start(sb[:], drm[:])
        nc.vector.memset(ones_sq[:], 1.0)

        L = persist.tile([KAUG, ROWS], bf16)
        Rk = [persist.tile([KAUG, 1024], bf16, name=f"R{k}") for k in range(NK)]
        nc.vector.memset(L[32:52, :], 1.0)
        for k in range(NK):
            nc.vector.memset(Rk[k][32:52, :], 1.0)

        inb = dram.tile([GROWS, ROWS], bf16)
        outb = dram.tile([NCORES * GROWS, ROWS], bf16)

        # ---------------- Phase 1: MLP on own cols -> splits ----------------
        CH = 512
        with tc.tile_pool(name="zpool", bufs=1) as zpool:
            zT = zpool.tile([EMB, ROWS], f32)
            zhi = zpool.tile([EMB, ROWS], bf16)
            zlo = zpool.tile([EMB, ROWS], bf16)
            sqh = zpool.tile([1, ROWS], bf16)
            sql = zpool.tile([1, ROWS], bf16)
            sp1 = zpool.tile([1, ROWS], f32)

            with (
                tc.tile_pool(name="mlp_h", bufs=2) as hpool,
                tc.tile_pool(name="ps1", bufs=2, space="PSUM") as ps1p,
                tc.tile_pool(name="ps2", bufs=2, space="PSUM") as ps2p,
                tc.tile_pool(name="ps3", bufs=2, space="PSUM") as ps3p,
                tc.tile_pool(name="ps4", bufs=1, space="PSUM") as ps4p,
                tc.tile_pool(name="pssq", bufs=1, space="PSUM") as psqp,
            ):
                NCH2 = ROWS // CH
                p1s = [ps1p.tile([64, CH], f32, name="p1") for n in range(NCH2)]
                h1s = [hpool.tile([64, CH], f32, name="h1") for n in range(NCH2)]
                p2s = [ps2p.tile([32, CH], f32, name="p2") for n in range(NCH2)]
                h2s = [hpool.tile([32, CH], f32, name="h2") for n in range(NCH2)]
                p3s = [ps3p.tile([16, CH], f32, name="p3") for n in range(NCH2)]
                h3s = [hpool.tile([16, CH], f32, name="h3") for n in range(NCH2)]
                for n in range(NCH2):
                    s = n * CH
                    nc.tensor.matmul(p1s[n][:], w1_sb[:], xt_sb[:, s:s + CH], start=True, stop=True)
                for n in range(NCH2):
                    nc.scalar.activation(h1s[n][:], p1s[n][:], AF.Relu, bias=b1_sb[:])
                    nc.tensor.matmul(p2s[n][:], w2_sb[:], h1s[n][:], start=True, stop=True)
                for n in range(NCH2):
                    nc.scalar.activation(h2s[n][:], p2s[n][:], AF.Relu, bias=b2_sb[:])
                    nc.tensor.matmul(p3s[n][:], w3_sb[:], h2s[n][:], start=True, stop=True)
                for n in range(NCH2):
                    s = n * CH
                    nc.scalar.activation(h3s[n][:], p3s[n][:], AF.Relu, bias=b3_sb[:])
                    p4 = ps4p.tile([16, CH], f32, name="p4")
                    nc.tensor.matmul(p4[:], w4_sb[:], h3s[n][:], start=True, stop=True)
                    nc.scalar.activation(zT[:, s:s + CH], p4[:], AF.Identity, bias=b4_sb[:])
                    nc.scalar.activation(zhi[:, s:s + CH], p4[:], AF.Identity, bias=b4_sb[:])
                    zt2 = hpool.tile([16, CH], f32, name="zt2")
                    nc.scalar.activation(zt2[:], p4[:], AF.Square, bias=b4_sb[:])
                    psq = psqp.tile([1, CH], f32, name="psq")
                    nc.tensor.matmul(psq[:], ones_sq[:], zt2[:], start=True, stop=True)
                    nc.vector.scalar_tensor_tensor(
                        zlo[:, s:s + CH], zT[:, s:s + CH], 0.0,
                        zhi[:, s:s + CH], Alu.add, Alu.subtract,
                    )
                    nc.scalar.activation(sqh[0:1, s:s + CH], psq[:], AF.Copy, bias=0.0)
                    nc.vector.scalar_tensor_tensor(
                        sql[0:1, s:s + CH], psq[:], 0.0,
                        sqh[0:1, s:s + CH], Alu.add, Alu.subtract,
                    )
                    nc.scalar.activation(sp1[0:1, s:s + CH], psq[:], AF.Copy, bias=1.0)
                    nc.sync.dma_start(inb[0:EMB, s:s + CH], zhi[:, s:s + CH])
                    nc.sync.dma_start(inb[EMB:2 * EMB, s:s + CH], zlo[:, s:s + CH])
                    nc.sync.dma_start(inb[32:33, s:s + CH], sqh[0:1, s:s + CH])
                    nc.sync.dma_start(inb[33:34, s:s + CH], sql[0:1, s:s + CH])

            # ---- gather hi/lo splits of all points ----
            nc.gpsimd.collective_compute(
                "AllGather",
                mybir.AluOpType.bypass,
                replica_groups=[list(range(NCORES))],
                ins=[inb.opt()],
                outs=[outb.opt()],
            )
            # scatter into R_k tiles: [0:16]=zhi, [16:32]=zhi dup,
            # [32:50]=zlo+sqh+sql (contiguous in outb), [50:52]=ones (memset)
            for c in (0, 4, 1, 5, 2, 6, 3, 7):
                nc.sync.dma_start(
                    Rk[c][0:EMB, :], outb[c * GROWS:c * GROWS + EMB, :]
                )
                nc.scalar.dma_start(
                    Rk[c][EMB:2 * EMB, :], outb[c * GROWS:c * GROWS + EMB, :]
                )
                nc.gpsimd.dma_start(
                    Rk[c][32:50, :], outb[c * GROWS + EMB:c * GROWS + GROWS, :]
                )

            # ---- build L from own-col splits ----
            with tc.tile_pool(name="fin", bufs=1) as fin:
                m2zhi = fin.tile([EMB, ROWS], bf16)
                m2zlo = fin.tile([EMB, ROWS], bf16)
                sph = fin.tile([1, ROWS], bf16)
                spl = fin.tile([1, ROWS], bf16)

                nc.scalar.activation(m2zhi[:], zhi[:, :], AF.Copy, bias=0.0, scale=-2.0)
                nc.scalar.activation(m2zlo[:], zlo[:, :], AF.Copy, bias=0.0, scale=-2.0)
                nc.scalar.activation(sph[:], sp1[:], AF.Copy, bias=0.0)
                nc.vector.scalar_tensor_tensor(
                    spl[:], sp1[:], 0.0, sph[:], Alu.add, Alu.subtract
                )
                nc.sync.dma_start(L[0:EMB, :], m2zhi[:])
                nc.sync.dma_start(L[EMB:2 * EMB, :], m2zlo[:])
                nc.sync.dma_start(L[32:48, :], m2zhi[:])
                nc.sync.dma_start(L[50:51, :], sph[:])
                nc.sync.dma_start(L[51:52, :], spl[:])

        # ------- Phase 2: dual-path 1/(1+d) -> rowsum -> normalize -> out -------
        NB = ROWS // 128
        chunks = []
        col = 0
        while col < N:
            k, off = col // 1024, col % 1024
            w = min(1024 - off, (CA - col) if col < CA else (N - col))
            w = min(w, 1024)
            chunks.append((k, off, w, col < CA))
            col += w
        n_dve = sum(1 for c in chunks if not c[3])
        NPART = 1 + n_dve
        a_chunks = [c for c in chunks if c[3]]
        d_chunks = [c for c in chunks if not c[3]]
        chunks = []
        for i in range(max(len(a_chunks), len(d_chunks))):
            if i < len(a_chunks):
                chunks.append(a_chunks[i])
            if i < len(d_chunks):
                chunks.append(d_chunks[i])

        with (
            tc.tile_pool(name="tpool", bufs=2) as tpool,
            tc.tile_pool(name="numh", bufs=4) as numhp,
            tc.tile_pool(name="rs", bufs=3) as rspool,
            tc.tile_pool(name="psA", bufs=2, space="PSUM") as psap,
            tc.tile_pool(name="psD", bufs=2, space="PSUM") as psdp,
        ):
            prev = None
            for m in range(NB):
                lm = L[:, m * 128:(m + 1) * 128]
                numh = numhp.tile([128, N], f16, name="numh")
                t = tpool.tile([128, CA], f32, name="t")
                rs = rspool.tile([128, NPART], f32, name="rs")
                junk = rspool.tile([128, NPART], f32, name="junk")
                rsum = rspool.tile([128, 1], f32, name="rsum")
                sden = rspool.tile([128, 1], f32, name="sden")

                if prev is not None:
                    pnumh, psden, pm = prev
                    nc.vector.tensor_scalar(
                        pnumh[:], pnumh[:], psden[:], None, Alu.mult
                    )
                    nc.sync.dma_start(out[pm * 128:(pm + 1) * 128, :], pnumh[:])
                di = 0
                for (k, off, w, is_act) in chunks:
                    col = k * 1024 + off
                    if is_act:
                        pa = psap.tile([128, w], f32, name="pa")
                        h0 = 0
                        while h0 < w:
                            hw = min(512, w - h0)
                            nc.tensor.matmul(
                                pa[:, h0:h0 + hw], lm,
                                Rk[k][:, off + h0:off + h0 + hw],
                                start=True, stop=True,
                            )
                            h0 += hw
                        nc.scalar.activation(t[:, col:col + w], pa[:], AF.Ln, bias=0.0)
                    else:
                        pd = psdp.tile([128, w], f32, name="pd")
                        h0 = 0
                        while h0 < w:
                            hw = min(512, w - h0)
                            nc.tensor.matmul(
                                pd[:, h0:h0 + hw], lm,
                                Rk[k][:, off + h0:off + h0 + hw],
                                start=True, stop=True,
                            )
                            h0 += hw
                        nc.vector._custom_dve(
                            recip_op,
                            out=numh[:, col:col + w],
                            in0=pd[:],
                            s0=RECIP_C0, s1=RECIP_C1, imm2=RECIP_C2,
                            accum_out=rs[:, 1 + di:2 + di],
                        )
                        di += 1

                nc.scalar.activation(
                    numh[:, 0:CA], t[:], AF.Exp, bias=0.0, scale=-1.0,
                    accum_out=rs[:, 0:1],
                )
                nc.scalar.activation(
                    junk[:], rs[:], AF.Copy, bias=0.0, accum_out=rsum[:]
                )
                nc.vector.reciprocal_approx_fast(sden[:], rsum[:])
                prev = (numh, sden, m)
            pnumh, psden, pm = prev
            nc.vector.tensor_scalar(pnumh[:], pnumh[:], psden[:], None, Alu.mult)
            nc.sync.dma_start(out[pm * 128:(pm + 1) * 128, :], pnumh[:])

    nc.compile()
    return nc


def _get_nc():
    if "nc" not in _CACHE:
        _CACHE["nc"] = _build_program()
    return _CACHE["nc"]


def make_in_maps(inputs):
    x = np.asarray(inputs["x"], dtype=np.float32)
    com = {
        "W1": np.ascontiguousarray(np.asarray(inputs["W1"], dtype=np.float32)),
        "W2": np.ascontiguousarray(np.asarray(inputs["W2"], dtype=np.float32)),
        "W3": np.ascontiguousarray(np.asarray(inputs["W3"], dtype=np.float32)),
        "W4": np.ascontiguousarray(np.asarray(inputs["W4"], dtype=np.float32)),
        "b1": np.ascontiguousarray(np.asarray(inputs["b1"], dtype=np.float32).reshape(-1, 1)),
        "b2": np.ascontiguousarray(np.asarray(inputs["b2"], dtype=np.float32).reshape(-1, 1)),
        "b3": np.ascontiguousarray(np.asarray(inputs["b3"], dtype=np.float32).reshape(-1, 1)),
        "b4": np.ascontiguousarray(np.asarray(inputs["b4"], dtype=np.float32).reshape(-1, 1)),
    }
    in_maps = []
    for c in range(NCORES):
        xT_c = np.ascontiguousarray(x[c * ROWS:(c + 1) * ROWS].T)
        in_maps.append({"xT": xT_c, **com})
    return in_maps


def run(inputs, trace=False):
    from concourse.bass_utils import run_bass_kernel_spmd

    nc = _get_nc()
    in_maps = make_in_maps(inputs)
    res = run_bass_kernel_spmd(nc, in_maps, core_ids=list(range(NCORES)), trace=trace)
    full = np.concatenate(
        [res.results[c]["out"] for c in range(NCORES)], axis=0
    ).astype(np.float32)
    return full, res


def kernel(**inputs):
    full, _ = run(inputs, trace=False)
    return full
